# revision 34
# baseline (speedup 1.0000x reference)
# DGCNN graph-feature module on 8 Trainium2 NeuronCores — v3.
#
# Data-parallel over batch B=8 (one batch element per core). Per core:
#   - distance ranking nd[i,j] = p_i.p_j - |p_j|^2/2 via THREE fp16 PE
#     matmuls (hi/lo split of p: hi*hi + lo*hi + hi*lo accumulate in PSUM;
#     norms row split hi/lo the same way) -> near-fp32 ranking accuracy
#   - exact top-16 per 128-point row-block via DVE max8/max_index/match_replace
#   - edge-MLP folded into point space: h1 = G[:, idx] + Cc[:, n] with
#     G = W1a p^T, Cc = (W1b - W1a) p^T
#   - gather: G packed as bf16 channel-pairs in fp32 words (channels c and
#     c+32 share a word), 4 copies on 128 partitions -> one ap_gather call
#     serves 4 blocks with 4 independent 2048-index lists (ap_gather cost
#     is ~27ns per per-core index, so packing halves the gather floor)
#   - unpack via small SBUF->SBUF DMAs into channel-stacked bf16 layout
#     (two blocks share 128 partitions), h1 kept SBUF-resident in fp16
#   - exact BatchNorm batch stats across 8 cores via tiny AllReduces
#   - max-pool over K via fp16 TT tree; pool commutes with BN2+ReLU
import numpy as np
from contextlib import ExitStack

import concourse.bass as bass
from concourse import bacc, library_config
import concourse.tile as tile
from concourse import mybir
from concourse.bass_utils import run_bass_kernel_spmd

B, N, C, K = 8, 4096, 64, 16
NB = N // 128                      # 32 row-blocks of 128 points
NCALL = NB // 4                    # 8 gather calls (4 blocks each)
M_TOTAL = float(B * N * K)         # BN sample count over the whole batch
EPS = 1e-5
NEG_BIG = -1e30
F32 = mybir.dt.float32
F16 = mybir.dt.float16
BF16 = mybir.dt.bfloat16
I16 = mybir.dt.int16
I32 = mybir.dt.int32
U32 = mybir.dt.uint32
AF = mybir.ActivationFunctionType
ALU = mybir.AluOpType
AX = mybir.AxisListType

_NC_CACHE = {}


def build_nc(n_cores=8):
    nc = bacc.Bacc("TRN2", target_bir_lowering=False, debug=False, num_devices=n_cores)
    pts = nc.declare_dram_parameter("pts", [N, C], F32, isOutput=False)
    w1aT = nc.declare_dram_parameter("w1aT", [C, C], F16, isOutput=False)
    w1cQlo = nc.declare_dram_parameter("w1cQlo", [C, 2 * C], F16, isOutput=False)
    w1cQhi = nc.declare_dram_parameter("w1cQhi", [C, 2 * C], F16, isOutput=False)
    w2dT = nc.declare_dram_parameter("w2dT", [2 * C, 2 * C], F16, isOutput=False)
    gb2 = nc.declare_dram_parameter("gb2", [2 * C, 4], F32, isOutput=False)
    foldm = nc.declare_dram_parameter("foldm", [2 * C, C], F32, isOutput=False)
    foldqlo = nc.declare_dram_parameter("foldqlo", [2 * C, C], F32, isOutput=False)
    foldqhi = nc.declare_dram_parameter("foldqhi", [2 * C, C], F32, isOutput=False)
    iotam = nc.declare_dram_parameter("iotam", [128, N], I32, isOutput=False)
    dupm = nc.declare_dram_parameter("dupm", [C, 2 * C], F32, isOutput=False)
    out_t = nc.declare_dram_parameter("out_t", [C, N], F32, isOutput=True)

    group = [list(range(n_cores))]

    with tile.TileContext(nc) as tc:
      with ExitStack() as ctx:
        per = ctx.enter_context(tc.tile_pool(name="per", bufs=1))
        small = ctx.enter_context(tc.tile_pool(name="small", bufs=3))
        dram = ctx.enter_context(tc.tile_pool(name="dram", bufs=1, space="DRAM"))

        # ---- persistent SBUF tensors
        TLhi = per.tile([C + 1, N], F16)    # rows 0..63 p_hi, row 64 ones
        TRa = per.tile([C + 1, N], F16)     # rows 0..63 p_hi, row 64 n_hi
        TRlo = per.tile([C + 1, N], F16)    # rows 0..63 p_lo, row 64 n_lo
        Gp4 = per.tile([2 * C, N], F32)     # packed G (bf16 pairs), 4 copies
        # CcQ4lo[32q+p, m] = Cc[p, m+128q]; CcQ4hi same for channels 32..63
        CcQ4lo = per.tile([2 * C, N], BF16)
        CcQ4hi = per.tile([2 * C, N], BF16)
        h1st = per.tile([2 * C, N * K // 2], F16)   # SBUF-resident h1
        s1cols = per.tile([2 * C, 2 * NCALL], F32)
        q1cols = per.tile([2 * C, 2 * NCALL], F32)
        s2cols = per.tile([2 * C, 8 * NCALL], F32)
        q2cols = per.tile([2 * C, NCALL], F32)

        # ---- DRAM collective bounce buffers
        cc1_in = dram.tile([C, 2], F32)
        cc1_out = dram.tile([C, 2], F32)
        cc1b_in = dram.tile([C, 2], F32)
        cc1b_out = dram.tile([C, 2], F32)
        cc2_in = dram.tile([C, 2], F32)
        cc2_out = dram.tile([C, 2], F32)
        cc2b_in = dram.tile([C, 2], F32)
        cc2b_out = dram.tile([C, 2], F32)

        # ---- constants
        identity = per.tile([128, 128], F32)
        ones128 = per.tile([128, 128], F32)
        nc.vector.memset(ones128, 1.0)
        nc.gpsimd.affine_select(
            identity, ones128, pattern=[[-1, 128]], compare_op=ALU.is_equal,
            fill=0.0, base=0, channel_multiplier=1,
        )
        # rep2[p, a, b] = 1 if b == p else 0   (shape [16, 2, 16]) —
        # wraps a block's transposed idx into 2x16 partitions (two Q7 cores)
        rep2 = per.tile([16, 2, 16], F32)
        nc.gpsimd.affine_select(
            rep2, ones128[0:16, 0:32].rearrange("p (a b) -> p a b", b=16),
            pattern=[[0, 2], [-1, 16]], compare_op=ALU.is_equal,
            fill=0.0, base=0, channel_multiplier=1,
        )
        ones_col = per.tile([C, 1], F32)
        nc.vector.memset(ones_col, 1.0)
        eps_col = per.tile([C, 1], F32)
        nc.vector.memset(eps_col, EPS)
        # encoded-topk constants: y = 64*nd + 10240 quantizes the ranking to
        # 1/64 with k in [0,16384); enc = (round(y)<<12) | col_index packs the
        # index into the low bits so FIND_INDEX8 passes are never needed
        b10240 = per.tile([128, 1], F32)
        nc.vector.memset(b10240, 10240.0)
        sh12 = per.tile([128, 1], I32)
        nc.vector.memset(sh12, 12)
        msk12 = per.tile([128, 1], I32)
        nc.vector.memset(msk12, 4095)


        w1aT_s = per.tile([C, C], F16)
        nc.sync.dma_start(out=w1aT_s, in_=w1aT[:, :])
        w1cQlo_s = per.tile([C, 2 * C], F16)
        nc.sync.dma_start(out=w1cQlo_s, in_=w1cQlo[:, :])
        w1cQhi_s = per.tile([C, 2 * C], F16)
        nc.sync.dma_start(out=w1cQhi_s, in_=w1cQhi[:, :])
        w2dT_s = per.tile([2 * C, 2 * C], F16)
        nc.sync.dma_start(out=w2dT_s, in_=w2dT[:, :])
        gb2_s = per.tile([2 * C, 4], F32)
        nc.sync.dma_start(out=gb2_s, in_=gb2[:, :])
        foldm_s = per.tile([2 * C, C], F32)
        nc.sync.dma_start(out=foldm_s, in_=foldm[:, :])
        foldqlo_s = per.tile([2 * C, C], F32)
        nc.sync.dma_start(out=foldqlo_s, in_=foldqlo[:, :])
        foldqhi_s = per.tile([2 * C, C], F32)
        nc.sync.dma_start(out=foldqhi_s, in_=foldqhi[:, :])
        dupm_s = per.tile([C, 2 * C], F32)
        nc.sync.dma_start(out=dupm_s, in_=dupm[:, :])

        # ================= PHASE A: transpose, hi/lo split, norms, G/Cc ===
        with tc.tile_pool(name="psA", bufs=2, space="PSUM") as psA, \
             tc.tile_pool(name="ldA", bufs=3) as ldA, \
             tc.tile_pool(name="sqA", bufs=1) as sqA:
            TLlo = sqA.tile([C + 1, N], F16, tag="TLlo")  # phase-A only
            for t in range(NB):
                sl = slice(t * 128, (t + 1) * 128)
                pt_tile = ldA.tile([128, C], F32)
                nc.sync.dma_start(out=pt_tile, in_=pts[sl, :])
                ps_tr = psA.tile([C, 128], F32)
                nc.tensor.transpose(ps_tr, pt_tile, identity)
                nc.scalar.activation(TLhi[0:C, sl], ps_tr, AF.Copy)
                nc.vector.tensor_sub(TLlo[0:C, sl], ps_tr, TLhi[0:C, sl])
            nc.vector.memset(TLhi[C:C + 1, :], 1.0)
            nc.vector.memset(TLlo[C:C + 1, :], 0.0)
            nc.scalar.activation(TRa[0:C, :], TLhi[0:C, :], AF.Copy)
            nc.scalar.activation(TRlo[0:C, :], TLlo[0:C, :], AF.Copy)

            # norms: n = -|p|^2/2, split hi/lo at base 0
            p32f = sqA.tile([C, N], F32, tag="p32f")
            nc.vector.tensor_add(p32f, TLhi[0:C, :], TLlo[0:C, :])
            sq64f = sqA.tile([C, N], F32, tag="sq64f")
            nc.vector.tensor_mul(sq64f, p32f, p32f)
            for j in range(N // 512):
                js = slice(j * 512, (j + 1) * 512)
                sq64c = sq64f[:, js]
                ps_row = psA.tile([1, 512], F32, tag="ps_row", bufs=2)
                nc.tensor.matmul(ps_row, lhsT=ones_col, rhs=sq64c,
                                 start=True, stop=True)
                n32c = sqA.tile([1, 512], F32, tag="n32c", bufs=2)
                nc.scalar.activation(n32c, ps_row, AF.Copy, scale=-0.5)
                nhi0c = sqA.tile([1, 512], F16, tag="nhi0c", bufs=2)
                nc.scalar.activation(nhi0c, n32c, AF.Copy)
                nloc = sqA.tile([1, 512], F16, tag="nloc", bufs=2)
                nc.vector.tensor_sub(nloc, n32c, nhi0c)
                nc.scalar.activation(TRa[C:C + 1, js], nhi0c, AF.Copy)
                nc.scalar.activation(TRlo[C:C + 1, js], nloc, AF.Copy)

            # G (plain, 64 rows, bf16) then packed into fp32 words;
            # Cc duplicated (lower plain, upper shifted -128 cols)
            Gbf = sqA.tile([C, N], BF16, tag="Gbf")
            for j in range(N // 512):
                js = slice(j * 512, (j + 1) * 512)
                ps_g = psA.tile([C, 512], F32, tag="ps_g")
                nc.tensor.matmul(ps_g, lhsT=w1aT_s, rhs=TLhi[0:C, js],
                                 start=True, stop=True)
                nc.scalar.activation(Gbf[:, js], ps_g, AF.Copy)
                ps_cl = psA.tile([2 * C, 512], F32, tag="ps_cl", bufs=1)
                nc.tensor.matmul(ps_cl, lhsT=w1cQlo_s, rhs=TLhi[0:C, js],
                                 start=True, stop=True)
                ps_ch = psA.tile([2 * C, 512], F32, tag="ps_ch", bufs=1)
                nc.tensor.matmul(ps_ch, lhsT=w1cQhi_s, rhs=TLhi[0:C, js],
                                 start=True, stop=True)
                for q in range(4):
                    qs = slice(32 * q, 32 * (q + 1))
                    lo = j * 512 - 128 * q
                    src0 = max(0, -lo)
                    nc.vector.tensor_copy(
                        CcQ4lo[qs, lo + src0:lo + 512],
                        ps_cl[qs, src0:512])
                    nc.scalar.activation(
                        CcQ4hi[qs, lo + src0:lo + 512],
                        ps_ch[qs, src0:512], AF.Copy)
            # pack: word p of Gp4 = (lo=G[p], hi=G[p+32]); then duplicate
            # across the four 32-partition quadrants
            gview = Gp4.bitcast(BF16).rearrange("p (j i) -> p i j", i=2)
            ghi_sh = sqA.tile([32, N], BF16, tag="ghi_sh")
            nc.sync.dma_start(out=ghi_sh, in_=Gbf[32:64, :])
            nc.vector.tensor_copy(gview[0:32, 0, :], Gbf[0:32, :])
            nc.vector.tensor_copy(gview[0:32, 1, :], ghi_sh)
            for q in range(1, 4):
                nc.sync.dma_start(out=Gp4[32 * q:32 * (q + 1), :],
                                  in_=Gp4[0:32, :])

        nc.gpsimd.load_library(library_config.ap_gather)

        def fold_reduce(scols, qcols, cc_in, cc_out, tg, quad=False):
            # quad=False: cols in h1st layout (fold partition c with c+64,
            # matrix foldm). quad=True: interleaved (lo,hi) col pairs in the
            # 4x32-quadrant layout (channel c lives on partitions 32q+c for
            # the lo cols and maps to c+32... via foldqlo/foldqhi).
            with tc.tile_pool(name="psC" + tg, bufs=1, space="PSUM") as psC:
                ps_f = psC.tile([C, 2], F32, name="ps_f")
                if quad:
                    ng = scols.shape[1] // 2
                    sv = scols.rearrange("c (g two) -> c two g", two=2)
                    qv = qcols.rearrange("c (g two) -> c two g", two=2)
                    sqlo = small.tile([2 * C, 2], F32, tag="sqlo" + tg)
                    sqhi = small.tile([2 * C, 2], F32, tag="sqhi" + tg)
                    nc.vector.reduce_sum(out=sqlo[:, 0:1], in_=sv[:, 0, :],
                                         axis=AX.X)
                    nc.vector.reduce_sum(out=sqlo[:, 1:2], in_=qv[:, 0, :],
                                         axis=AX.X)
                    nc.vector.reduce_sum(out=sqhi[:, 0:1], in_=sv[:, 1, :],
                                         axis=AX.X)
                    nc.vector.reduce_sum(out=sqhi[:, 1:2], in_=qv[:, 1, :],
                                         axis=AX.X)
                    nc.tensor.matmul(ps_f, lhsT=foldqlo_s, rhs=sqlo,
                                     start=True, stop=False)
                    nc.tensor.matmul(ps_f, lhsT=foldqhi_s, rhs=sqhi,
                                     start=False, stop=True)
                else:
                    sq2 = small.tile([2 * C, 2], F32, tag="sq2_f" + tg)
                    nc.vector.reduce_sum(out=sq2[:, 0:1], in_=scols,
                                         axis=AX.X)
                    nc.vector.reduce_sum(out=sq2[:, 1:2], in_=qcols,
                                         axis=AX.X)
                    nc.tensor.matmul(ps_f, lhsT=foldm_s, rhs=sq2,
                                     start=True, stop=True)
                sq1 = small.tile([C, 2], F32, tag="sq_fold" + tg, name="sq1")
                nc.scalar.activation(sq1, ps_f, AF.Copy)
            nc.sync.dma_start(out=cc_in[:], in_=sq1)
            nc.gpsimd.collective_compute(
                "AllReduce", ALU.add, replica_groups=group,
                ins=[cc_in[:].opt()], outs=[cc_out[:].opt()],
            )
            st = small.tile([C, 2], F32, tag="st_in" + tg, name="st")
            nc.sync.dma_start(out=st, in_=cc_out[:])
            return st

        def stats_to_affine_dup(st, g_col, b_col):
            mean = small.tile([C, 1], F32, tag="mean")
            nc.vector.tensor_scalar_mul(mean, st[:, 0:1], 1.0 / M_TOTAL)
            ex2 = small.tile([C, 1], F32, tag="ex2")
            nc.vector.tensor_scalar_mul(ex2, st[:, 1:2], 1.0 / M_TOTAL)
            m2 = small.tile([C, 1], F32, tag="m2")
            nc.vector.tensor_mul(m2, mean, mean)
            var = small.tile([C, 1], F32, tag="var")
            nc.vector.tensor_sub(var, ex2, m2)
            sd = small.tile([C, 1], F32, tag="sd")
            nc.scalar.activation(sd, var, AF.Sqrt, bias=eps_col)
            rs = small.tile([C, 1], F32, tag="rs")
            nc.vector.reciprocal(rs, sd)
            a = small.tile([C, 1], F32, tag="a_aff")
            nc.vector.tensor_mul(a, g_col, rs)
            tmp = small.tile([C, 1], F32, tag="tmp_aff")
            nc.vector.tensor_mul(tmp, mean, a)
            b = small.tile([C, 1], F32, tag="b_aff")
            nc.vector.tensor_sub(b, b_col, tmp)
            ab = small.tile([C, 2], F32, tag="ab_cat")
            nc.vector.tensor_copy(ab[:, 0:1], a)
            nc.vector.tensor_copy(ab[:, 1:2], b)
            with tc.tile_pool(name="psCd", bufs=1, space="PSUM") as psCd:
                ps_d = psCd.tile([2 * C, 2], F32)
                nc.tensor.matmul(ps_d, lhsT=dupm_s, rhs=ab,
                                 start=True, stop=True)
                ab2 = small.tile([2 * C, 2], F32, tag="ab2")
                nc.scalar.activation(ab2, ps_d, AF.Copy)
            return ab2

        # defs hoisted so phase B can emit the partial AllReduce
        # ================= PHASE B: distances, top-16, packed gather ======
        with tc.tile_pool(name="psB", bufs=4, space="PSUM") as psB, \
             tc.tile_pool(name="psBs", bufs=2, space="PSUM") as psBs, \
             tc.tile_pool(name="ndb", bufs=1) as ndb, \
             tc.tile_pool(name="ghb", bufs=2) as ghb, \
             tc.tile_pool(name="ghu", bufs=1) as ghu, \
             tc.tile_pool(name="ixb", bufs=2) as ixb, \
             tc.tile_pool(name="encp", bufs=1) as encp, \
             tc.tile_pool(name="scrb", bufs=1) as scrb:

            iotam_s = encp.tile([128, N], I32, tag="iotam")
            nc.sync.dma_start(out=iotam_s, in_=iotam[:, :])
            enci = encp.tile([128, N], I32, tag="enci")

            state = {}

            def emit_dist(t):
                # PE matmuls (hi/lo) + Act PSUM->SBUF copies -> nd for block t
                sl = slice(t * 128, (t + 1) * 128)
                nd = ndb.tile([128, N], F32)
                state[("nd", t)] = nd
                for j in range(N // 512):
                    js = slice(j * 512, (j + 1) * 512)
                    # 2 matmuls: hi*(hi+lo) both sides; the dropped lo_i*hi_j
                    # term (~2e-3) is far below the 1/64 ranking quantum
                    ps_nd = psB.tile([128, 512], F32)
                    nc.tensor.matmul(ps_nd, lhsT=TLhi[:, sl],
                                     rhs=TRa[:, js], start=True, stop=False)
                    nc.tensor.matmul(ps_nd, lhsT=TLhi[:, sl],
                                     rhs=TRlo[:, js], start=False, stop=True)
                    nc.scalar.activation(nd[:, js], ps_nd, AF.Identity,
                                         scale=64.0, bias=b10240[:, 0:1])

            def emit_topk(t):
                # DVE-only encoded top-16: cast y=64*nd+10240 to int (round),
                # enc = (k<<12)|j, then 8-way split MAX8 (exact unless one
                # eighth holds >8 of the top-16: P ~ 3e-4/row) and a tiny
                # 64-candidate merge; indices come from the low 12 bits, so
                # no FIND_INDEX8 / full-width MATCH_REPLACE8 passes at all.
                y = state.pop(("nd", t))
                nc.vector.tensor_copy(enci, y)
                nc.vector.scalar_tensor_tensor(
                    out=y.bitcast(I32), in0=enci, scalar=sh12[:, 0:1],
                    in1=iotam_s,
                    op0=ALU.logical_shift_left, op1=ALU.bitwise_or)
                encf = y
                cand = small.tile([128, 64], F32, tag="cand", bufs=1)
                for q in range(8):
                    nc.vector.max(out=cand[:, 8 * q:8 * (q + 1)],
                                  in_=encf[:, 512 * q:512 * (q + 1)])
                m8 = small.tile([128, 8], F32, tag="m8", bufs=1)
                nc.vector.max(out=m8, in_=cand)
                m16 = small.tile([128, 16], F32, tag="m16")
                nc.vector.tensor_copy(m16[:, 0:8], m8)
                nc.vector.match_replace(out=cand, in_to_replace=m8,
                                        in_values=cand, imm_value=0.0)
                nc.vector.max(out=m16[:, 8:16], in_=cand)
                idxi = small.tile([128, 16], I32, tag="idxi")
                nc.vector.tensor_scalar(out=idxi, in0=m16.bitcast(I32),
                                        scalar1=msk12[:, 0:1], scalar2=None,
                                        op0=ALU.bitwise_and)
                idxf = small.tile([128, 16], F32)
                nc.vector.tensor_copy(idxf, idxi)
                state[("idxf", t)] = idxf

            def emit_idx(t, ps_rep):
                # PE transpose + Act copy + PE rep-wrap of block t's indices
                idxf = state.pop(("idxf", t))
                ps_tr16 = psBs.tile([16, 128], F32, tag="ps_tr16", bufs=1)
                nc.tensor.transpose(ps_tr16, idxf, identity)
                idxTf = small.tile([16, 128], F32)
                nc.scalar.activation(idxTf, ps_tr16, AF.Copy)
                # wrap into the block's 32-partition quadrant (2 Q7 cores)
                q = t % 4
                nc.tensor.matmul(ps_rep[32 * q:32 * (q + 1), :], lhsT=rep2,
                                 rhs=idxTf, start=True, stop=True,
                                 tile_position=(0, 32 * q))

            def emit_gather(g):
                idxw = ixb.tile([2 * C, 128], I16, tag="idxw")
                nc.vector.tensor_copy(idxw, state.pop(("ps_rep", g)))
                gh = ghb.tile([2 * C, 2048], F32, tag="gh")
                state[("gh", g)] = gh
                nc.gpsimd.ap_gather(
                    out_ap=gh.rearrange("c (i o) -> c i o", o=1),
                    in_ap=Gp4.rearrange("c (i o) -> c i o", o=1),
                    idxs_ap=idxw, channels=2 * C, num_elems=N, d=1,
                    num_idxs=2048,
                )

            def emit_h1_stats(g):
                # h1 = gh(packed, strided read) + Cc(quad layout), built on
                # the otherwise-idle Pool engine with the BN1 sum fused in
                # via accum_out; sumsq likewise as (h1*1)*h1 on Pool. The
                # partition shift into h1st is 8 contiguous DMAs.
                gh = state.pop(("gh", g))
                ghv = gh.bitcast(BF16).rearrange("c (j i) -> c i j", i=2)
                base = slice(512 * g, 512 * g + 128)
                cclo = CcQ4lo[:, base].rearrange(
                    "c (n o) -> c n o", o=1).to_broadcast([2 * C, 128, K])
                cchi = CcQ4hi[:, base].rearrange(
                    "c (n o) -> c n o", o=1).to_broadcast([2 * C, 128, K])
                h1lo = ghu.tile([2 * C, 2048], F16, tag="h1lo")
                h1hi = ghu.tile([2 * C, 2048], F16, tag="h1hi")
                nc.vector.tensor_add(
                    h1lo.rearrange("c (n k) -> c n k", k=K),
                    ghv[:, 0, :].rearrange("c (n k) -> c n k", k=K), cclo)
                nc.vector.tensor_add(
                    h1hi.rearrange("c (n k) -> c n k", k=K),
                    ghv[:, 1, :].rearrange("c (n k) -> c n k", k=K), cchi)
                for q in range(4):
                    qsl = slice(32 * q, 32 * (q + 1))
                    pb = 64 * (q % 2)
                    csl = slice((2 * g + q // 2) * 2048,
                                (2 * g + q // 2) * 2048 + 2048)
                    nc.sync.dma_start(out=h1st[pb:pb + 32, csl],
                                      in_=h1lo[qsl, :])
                    nc.sync.dma_start(out=h1st[pb + 32:pb + 64, csl],
                                      in_=h1hi[qsl, :])
                for hh in range(2):
                    hsl = slice(g * 4096 + 2048 * hh,
                                g * 4096 + 2048 * (hh + 1))
                    scr = scrb.tile([2 * C, 2048], F16, tag="scr")
                    nc.scalar.activation(scr, h1st[:, hsl], AF.Copy,
                                         accum_out=s1cols[:, 2 * g + hh:
                                                          2 * g + hh + 1])
                    scr2 = scrb.tile([2 * C, 2048], F16, tag="scr")
                    nc.scalar.activation(scr2, h1st[:, hsl], AF.Square,
                                         accum_out=q1cols[:, 2 * g + hh:
                                                          2 * g + hh + 1])

            # 3-stage software pipeline: PE dist(t) runs ahead of DVE topk(t-1)
            # and PE idx-wrap(t-2) so no engine queue stalls on another's output
            for tt in range(NB + 2):
                if tt < NB:
                    if tt % 4 == 0:
                        ps_rep = psBs.tile([128, 128], F32, tag="ps_rep")
                        state[("ps_rep", tt // 4)] = ps_rep
                    emit_dist(tt)
                if 0 <= tt - 1 < NB:
                    emit_topk(tt - 1)
                if 0 <= tt - 2 < NB:
                    t2 = tt - 2
                    emit_idx(t2, state[("ps_rep", t2 // 4)])
                    if t2 % 4 == 3:
                        g = t2 // 4
                        emit_gather(g)
                        if g >= 2:
                            emit_h1_stats(g - 2)
            emit_h1_stats(NCALL - 2)
            st1a = fold_reduce(s1cols[:, 0:14], q1cols[:, 0:14],
                               cc1_in, cc1_out, "1a")
            emit_h1_stats(NCALL - 1)

        # ================= PHASE C: stats1 fold + allreduce -> a1, b1 =====
        latep = ctx.enter_context(tc.tile_pool(name="latep", bufs=1))
        pooledh = latep.tile([2 * C, N // 2], F16)
        st1b = fold_reduce(s1cols[:, 14:16], q1cols[:, 14:16],
                           cc1b_in, cc1b_out, "1b")
        st1 = small.tile([C, 2], F32, tag="st1_sum")
        nc.vector.tensor_add(st1, st1a, st1b)
        ab1 = stats_to_affine_dup(st1, gb2_s[0:C, 0:1], gb2_s[0:C, 1:2])

        # ================= PHASE D: z=relu(a1*h1+b1), h2=W2 z, pool, stats2
        with tc.tile_pool(name="psD", bufs=4, space="PSUM") as psD, \
             tc.tile_pool(name="zb", bufs=2) as zb:
            for g in range(NCALL):
                hsl = slice(g * 4096, (g + 1) * 4096)
                z = zb.tile([2 * C, 4096], F16, tag="z")
                nc.scalar.activation(z, h1st[:, hsl], AF.Relu,
                                     scale=ab1[:, 0:1], bias=ab1[:, 1:2])
                h2s = zb.tile([2 * C, 4096], F16, tag="h2s")
                for j in range(8):
                    js = slice(j * 512, (j + 1) * 512)
                    ps_h2 = psD.tile([2 * C, 512], F32)
                    nc.tensor.matmul(ps_h2, lhsT=w2dT_s, rhs=z[:, js],
                                     start=True, stop=True)
                    nc.scalar.activation(h2s[:, js], ps_h2, AF.Copy,
                                         accum_out=s2cols[:, 8 * g + j:
                                                          8 * g + j + 1])
                scr3 = zb.tile([2 * C, 4096], F16, tag="scr3", bufs=1)
                nc.scalar.activation(scr3, h2s, AF.Square,
                                     accum_out=q2cols[:, g:g + 1])
                # max over K=16 via fp16 TT tree on the idle Pool engine
                h2v = h2s.rearrange("c (n k) -> c n k", k=K)
                t1 = zb.tile([2 * C, 256, 8], F16, tag="t1")
                nc.vector.tensor_max(t1, h2v[:, :, 0:8], h2v[:, :, 8:16])
                t2 = zb.tile([2 * C, 256, 4], F16, tag="t2")
                nc.vector.tensor_max(t2, t1[:, :, 0:4], t1[:, :, 4:8])
                t3 = zb.tile([2 * C, 256, 2], F16, tag="t3")
                nc.vector.tensor_max(t3, t2[:, :, 0:2], t2[:, :, 2:4])
                nc.vector.tensor_max(
                    pooledh[:, g * 256:(g + 1) * 256].rearrange(
                        "c (n o) -> c n o", o=1),
                    t3[:, :, 0:1], t3[:, :, 1:2])
                if g == NCALL - 2:
                    st2a = fold_reduce(s2cols[:, 0:56], q2cols[:, 0:7],
                                       cc2_in, cc2_out, "2a")

        # ================= PHASE E: stats2 allreduce -> final =============
        st2b = fold_reduce(s2cols[:, 56:64], q2cols[:, 7:8],
                           cc2b_in, cc2b_out, "2b")
        st2 = small.tile([C, 2], F32, tag="st2_sum")
        nc.vector.tensor_add(st2, st2a, st2b)
        ab2 = stats_to_affine_dup(st2, gb2_s[0:C, 2:3], gb2_s[0:C, 3:4])

        with tc.tile_pool(name="feb", bufs=1) as feb:
            final = feb.tile([2 * C, N // 2], F32)
            nc.scalar.activation(final, pooledh, AF.Relu,
                                 scale=ab2[:, 0:1], bias=ab2[:, 1:2])
            out_v = out_t[:, :].rearrange("c (p t n) -> c p t n", t=2, n=128)
            nc.sync.dma_start(
                out=out_v[:, :, 0, :],
                in_=final[0:C, :].rearrange("c (p n) -> c p n", n=128))
            nc.sync.dma_start(
                out=out_v[:, :, 1, :],
                in_=final[C:2 * C, :].rearrange("c (p n) -> c p n", n=128))

    nc.finalize()
    return nc


def _get_nc(n_cores=8):
    if n_cores not in _NC_CACHE:
        _NC_CACHE[n_cores] = build_nc(n_cores)
    return _NC_CACHE[n_cores]


def make_in_maps(points, W1, gamma1, beta1, W2, gamma2, beta2, n_cores=8):
    pts = np.ascontiguousarray(np.asarray(points, np.float32))
    W1 = np.asarray(W1, np.float32)
    w1a = W1[:, :C]
    w1c = W1[:, C:] - W1[:, :C]
    w1aT = np.ascontiguousarray(w1a.T.astype(np.float16))
    w1cQlo = np.ascontiguousarray(
        np.tile(w1c.T[:, 0:32], (1, 4)).astype(np.float16))
    w1cQhi = np.ascontiguousarray(
        np.tile(w1c.T[:, 32:64], (1, 4)).astype(np.float16))
    W2f = np.asarray(W2, np.float32)
    w2d = np.zeros((2 * C, 2 * C), np.float32)
    w2d[:C, :C] = W2f.T
    w2d[C:, C:] = W2f.T
    w2dT = np.ascontiguousarray(w2d.astype(np.float16))
    gb = np.stack([np.asarray(gamma1, np.float32), np.asarray(beta1, np.float32),
                   np.asarray(gamma2, np.float32), np.asarray(beta2, np.float32)],
                  axis=1)
    gb2 = np.ascontiguousarray(np.concatenate([gb, gb], axis=0))
    foldm = np.zeros((2 * C, C), np.float32)
    foldm[:C, :] = np.eye(C, dtype=np.float32)
    foldm[C:, :] = np.eye(C, dtype=np.float32)
    dupm = np.ascontiguousarray(foldm.T.copy())
    # quad-layout folds: channel c of the lo(0:32)/hi(32:64) halves lives on
    # partitions 32q+c, q=0..3
    foldqlo = np.zeros((2 * C, C), np.float32)
    foldqhi = np.zeros((2 * C, C), np.float32)
    for q in range(4):
        for c in range(32):
            foldqlo[32 * q + c, c] = 1.0
            foldqhi[32 * q + c, 32 + c] = 1.0
    iotam = np.ascontiguousarray(
        np.broadcast_to(np.arange(N, dtype=np.int32), (128, N)).copy())
    return [
        {"pts": np.ascontiguousarray(pts[b]), "w1aT": w1aT,
         "w1cQlo": w1cQlo, "w1cQhi": w1cQhi,
         "w2dT": w2dT, "gb2": gb2, "foldm": foldm,
         "foldqlo": foldqlo, "foldqhi": foldqhi, "dupm": dupm,
         "iotam": iotam}
        for b in range(n_cores)
    ]


def kernel(points, W1, gamma1, beta1, W2, gamma2, beta2, **run_kwargs):
    nc = _get_nc(B)
    in_maps = make_in_maps(points, W1, gamma1, beta1, W2, gamma2, beta2, B)
    res = run_bass_kernel_spmd(nc, in_maps, core_ids=list(range(B)), **run_kwargs)
    out = np.stack([np.asarray(res.results[b]["out_t"]).T for b in range(B)],
                   axis=0)
    kernel.last_results = res
    return out.astype(np.float32)

